# revision 32
# baseline (speedup 1.0000x reference)
"""CopyGenerator kernel for 8 Trainium2 NeuronCores.

Strategy: tensor-parallel over the vocab dimension, collective-free,
fp8 (e4m3) DoubleRow matmul, with the PSUM drain split across two
engines so the PE is the pacer.
  - Each core computes logits = hidden @ W[:, k*4000:(k+1)*4000] as an fp8
    DoubleRow matmul (2 contraction rows per PE cell, fp32 accumulate;
    host pre-scales h by 16 and W by 1024), in 2000-col stripes
    ([128, 4, 512] PSUM tiles, 4 banks, double buffered).
  - Drain split: each phase-2 stripe is drained by BOTH engines at once:
    the scalar engine ACT-exps chunks 0-1 (with scale 1/16384 and
    per-row bias ln(1-p_copy), emitting e = exp(logit)*(1-p_copy) in
    bf16) while the otherwise idle DVE raw-copies chunks 2-3 (bf16
    scaled logits, exp'd on the host). Phase-1 stripes (1000 cols,
    input-DMA paced) alternate whole-stripe drains between the two
    engines. DRAIN_TAB records which 1000-col blocks are raw.
  - No AllReduce: the host finishes the softmax denominator and applies
    the 1/Z row scale while upcasting the bf16 shards to fp32.
  - p_copy = sigmoid(hidden @ Wc + bc) is computed on the host; the
    device receives ln(1-p_copy) as an ACT bias and a pre-scaled
    attention (attn * p_copy) for the copy path.
  - Copy path (einsum over src_map, bf16) sharded 4 batches per core,
    placed at the end of phase 1 (sharing the stripe PSUM pool pins its
    schedule slot) after its inputs have streamed in on the SWDGE ring.
  - Output DMAs alternate between the sync and scalar HWDGE queues.
Host side: shard/cast/pack inputs, run SPMD on cores 0-7, exp the raw
blocks, normalize + gather.
"""

import numpy as np
import ml_dtypes

bf16 = ml_dtypes.bfloat16
f8 = ml_dtypes.float8_e4m3

# Problem shape (hardcoded per contract)
B, T, S, C, D, V = 32, 64, 400, 100, 512, 32000
R = B * T              # 2048 rows, row r = t*32 + b
NC = 8
VS = V // NC           # 4000 vocab cols per core
PAD_IDX = 1
NEG_INF = -1e9

KCH = D // 128         # 4 contraction chunks of 128
NRB = R // 128         # 16 row blocks
SCH = 4                # s-chunks of 100 for the copy einsum
NW = 3                 # row blocks covered by phase 1
OUT_BUFS = 6
SH = 16.0              # host pre-scale on hidden (fp8 range)
SW = 1024.0            # host pre-scale on W (fp8 range)
# hT row-chunks (DMA granules); each stored chunk-contiguous per partition
HT_CH = [(0, 384), (384, 512), (512, 1024), (1024, 1536), (1536, 2048)]
HT_OFF = [0]
for _r0, _r1 in HT_CH:
    HT_OFF.append(HT_OFF[-1] + KCH * (_r1 - _r0))


def _dve_blocks(rb):
    """Which 500-col blocks of row-block rb hold raw scaled logits
    (DVE drain) instead of exp'd values (ACT drain). Must match the
    device schedule below."""
    if rb < NW:
        return [b for qp in range(4) if (3 * qp + rb) % 2 == 1
                for b in (2 * qp, 2 * qp + 1)]
    if rb == NRB - 1:
        return [2, 3, 5, 7]
    return [2, 3, 6, 7]


_cache = {}


def _build(all_bias: bool):
    import concourse.bass as bass
    import concourse.mybir as mybir
    import concourse.tile as tile
    from concourse import bacc

    fp32 = mybir.dt.float32
    bf = mybir.dt.bfloat16
    f8d = mybir.dt.float8e4
    AF = mybir.ActivationFunctionType
    DR = mybir.MatmulPerfMode.DoubleRow

    nc = bacc.Bacc("TRN2", target_bir_lowering=False, debug=False, num_devices=NC)

    # ---- I/O ----
    # hTp: packed [128, kch*rows] per row-chunk; Wp: packed [128, q, kch, 512]
    hT_d = nc.dram_tensor("hTp", [128, KCH * R], f8d, kind="ExternalInput")
    W_d = nc.dram_tensor("Wp", [128, 8 * KCH * 512], f8d, kind="ExternalInput")
    lnb_d = nc.dram_tensor("lnb", [128, NRB], fp32, kind="ExternalInput")
    cpin_d = nc.dram_tensor("cpin", [100, SCH * (256 + 4 * C)], bf,
                            kind="ExternalInput")
    out_d = nc.dram_tensor("out", [R, VS], bf, kind="ExternalOutput")
    cp_d = nc.dram_tensor("cp", [T, 4 * C], fp32, kind="ExternalOutput")
    if all_bias:
        bias_d = nc.dram_tensor("biask", [1, VS], bf, kind="ExternalInput")

    with tile.TileContext(nc) as tc:
        with (
            tc.tile_pool(name="sb", bufs=1) as sb,
            tc.tile_pool(name="ps", bufs=4, space="PSUM") as ps,
        ):
            # ---- resident loads ----
            # sync ring: hT head + all W chunk-pairs; gpsimd ring (parallel):
            # lnb, copy-path inputs, then the hT bulk. All transfers are
            # fully contiguous on both sides (host packs the layouts).
            hT_ch = [sb.tile([128, KCH, r1 - r0], f8d, name=f"hT{ci}")
                     for ci, (r0, r1) in enumerate(HT_CH)]
            hT_view = hT_d.ap()
            W_sb = sb.tile([128, 8, KCH, 512], f8d)
            W_view = W_d.ap()

            def hT_dma(eng, ci):
                o0, o1 = HT_OFF[ci], HT_OFF[ci + 1]
                getattr(nc, eng).dma_start(hT_ch[ci][:, :, :], hT_view[:, o0:o1])

            def hT_op(rb, kp):
                """[128, 2, 128] DoubleRow stationary operand: row block rb,
                contraction rows kp*256..kp*256+255."""
                r = rb * 128
                for ci, (r0, r1) in enumerate(HT_CH):
                    if r0 <= r < r1:
                        return hT_ch[ci][:, 2 * kp:2 * kp + 2, r - r0:r - r0 + 128]
                raise AssertionError(rb)

            # warmup operand memsets go first on the gpsimd queue (ready
            # earliest; must precede its DMA issues in queue order)
            wu_w = sb.tile([128, 128], bf)
            wu_x = sb.tile([128, 512], bf)
            nc.gpsimd.memset(wu_w[:, :], 0.0)
            nc.gpsimd.memset(wu_x[:, :], 0.0)

            # Input load split across BOTH HWDGE rings (they serialize
            # per-DMA overheads internally, and the scalar ring is idle
            # until the first ACT at ~15us): W pairs 0-1 stream on the
            # scalar ring in parallel with hT0 + W pairs 2-3 + the hT bulk
            # on the sync ring, so the PE can start ~4us earlier
            for k in range(2):
                nc.scalar.dma_start(W_sb[:, 2 * k:2 * k + 2, :, :],
                                    W_view[:, k * 4096:(k + 1) * 4096])
            hT_dma("sync", 0)
            for k in range(2, 4):
                nc.sync.dma_start(W_sb[:, 2 * k:2 * k + 2, :, :],
                                  W_view[:, k * 4096:(k + 1) * 4096])
            for ci in range(1, 5):
                hT_dma("sync", ci)

            lnb_sb = sb.tile([128, NRB], fp32)
            nc.gpsimd.dma_start(lnb_sb[:, :], lnb_d.ap())
            cpin_sb = sb.tile([100, SCH, 256 + 4 * C], bf)
            nc.gpsimd.dma_start(cpin_sb[:, :, :], cpin_d.ap())
            if all_bias:
                bias_sb = sb.tile([1, VS], bf)
                nc.gpsimd.dma_start(bias_sb[:, :], bias_d.ap())
                ones_sb = sb.tile([1, 128], bf)
                nc.vector.memset(ones_sb[:, :], 1.0)

            cp_sb = sb.tile([64, 4 * C], fp32)

            ot_tiles = {}

            def get_ot(rb):
                if rb not in ot_tiles:
                    ot_tiles[rb] = sb.tile([128, VS], bf, tag="ot",
                                           bufs=OUT_BUFS, name=f"ot{rb}")
                return ot_tiles[rb]

            def drain(rb, st, c0, raw):
                """Move a [128, 2, 500] PSUM tile (cols c0*500..c0*500+999)
                into the out tile: ACT exp (raw=False) or DVE raw copy."""
                ot = get_ot(rb)
                ev = ot[:, c0 * 500:(c0 + 2) * 500].rearrange(
                    "p (g v) -> p g v", g=2)
                si = st[:, 0:2, 0:500]
                if raw:
                    nc.vector.tensor_copy(ev, si)
                else:
                    nc.scalar.activation(ev, si, AF.Exp, scale=1.0 / (SH * SW),
                                         bias=lnb_sb[:, rb:rb + 1])

            def mm(st, rb, c0, j, kp):
                nc.tensor.matmul(
                    st[:, j, 0:500],
                    hT_op(rb, kp),
                    W_sb[:, c0 + j, 2 * kp:2 * kp + 2, 0:500],
                    start=(kp == 0),
                    stop=(kp == 1 and not all_bias),
                    perf_mode=DR)

            def bias_mm(st, c0):
                for j in range(2):
                    nc.tensor.matmul(
                        st[:, j, 0:500], ones_sb[:, :],
                        bias_sb[:, (c0 + j) * 500:(c0 + j + 1) * 500],
                        start=False, stop=True)

            def stripe_mm(rb, c0):
                """1000-col stripe of DoubleRow matmuls; returns PSUM tile."""
                st = ps.tile([128, 2, 512], fp32, tag="stripe",
                             name=f"l{rb}_{c0}")
                for kp in range(2):
                    for j in range(2):
                        mm(st, rb, c0, j, kp)
                if all_bias:
                    bias_mm(st, c0)
                return st

            def half_mm(rb, c0):
                """2000-col half-row: two PSUM tiles (X: cols c0*500..+999,
                Y: +1000..+1999) with the matmuls interleaved so accumulation
                pairs are 3 apart (keeps the PE at full cadence)."""
                stx = ps.tile([128, 2, 512], fp32, tag="stripe",
                              name=f"lx{rb}_{c0}")
                sty = ps.tile([128, 2, 512], fp32, tag="stripe",
                              name=f"ly{rb}_{c0}")
                for kp in range(2):
                    for j in range(2):
                        mm(stx, rb, c0, j, kp)
                    for j in range(2):
                        mm(sty, rb, c0 + 2, j, kp)
                if all_bias:
                    bias_mm(stx, c0)
                    bias_mm(sty, c0 + 2)
                return stx, sty

            def emit_out(rb, c0, c1, eng):
                eng.dma_start(
                    out_d.ap()[rb * 128:(rb + 1) * 128, c0 * 500:c1 * 500],
                    ot_tiles[rb][:, c0 * 500:c1 * 500])

            # ---- PE warmup: dummy matmuls on zero tiles so the HAM clock
            # gate ramps while the input DMAs stream in ----
            wu_ps = ps.tile([128, 2, 512], fp32, tag="stripe", name="warm")
            for i in range(6):
                nc.tensor.matmul(wu_ps[:, 0, :], wu_w[:, :], wu_x[:, :],
                                 start=True, stop=True)

            # ---- phase 1: chunk-pair-major over rb0-2 (1000-col stripes)
            # so the PE starts as soon as the first W pair lands; paced by
            # the input DMA stream. Whole-stripe drains alternate between
            # ACT and DVE in issue order; the rb0/rb1 stripes are
            # MM-interleaved to keep accumulation pairs 3 apart ----
            for qp in range(4):
                sta = ps.tile([128, 2, 512], fp32, tag="stripe",
                              name=f"l0_{2 * qp}")
                stb = ps.tile([128, 2, 512], fp32, tag="stripe",
                              name=f"l1_{2 * qp}")
                for kp in range(2):
                    for j in range(2):
                        mm(sta, 0, 2 * qp, j, kp)
                    for j in range(2):
                        mm(stb, 1, 2 * qp, j, kp)
                if all_bias:
                    bias_mm(sta, 2 * qp)
                    bias_mm(stb, 2 * qp)
                drain(0, sta, 2 * qp, raw=(3 * qp) % 2 == 1)
                drain(1, stb, 2 * qp, raw=(3 * qp + 1) % 2 == 1)
                st = stripe_mm(2, 2 * qp)
                drain(2, st, 2 * qp, raw=(3 * qp + 2) % 2 == 1)
                if qp == 1:
                    for rb in range(NW):
                        emit_out(rb, 0, 4, nc.gpsimd)
            # copy path: cp[t, bb*C:(bb+1)*C] = sum_s attnT[s, bb*64+t]
            #   * srcmap[s, bb, :]  (attnT pre-scaled by p_copy on the host)
            cpps = ps.tile([64, 4 * C], fp32, tag="stripe", name="cpps")
            for bb in range(4):
                for c in range(SCH):
                    nc.tensor.matmul(
                        cpps[:, bb * C:(bb + 1) * C],
                        cpin_sb[:, c, bb * 64:(bb + 1) * 64],
                        cpin_sb[:, c, 256 + bb * C:256 + (bb + 1) * C],
                        start=(c == 0), stop=(c == SCH - 1))
            nc.vector.tensor_copy(cp_sb[:, :], cpps[:, :])
            nc.gpsimd.dma_start(cp_d.ap(), cp_sb[:, :])
            for rb in range(NW):
                emit_out(rb, 4, 8, nc.gpsimd)

            # ---- phase 2: row-major 2000-col half-rows, each drained by
            # BOTH engines at once on independent PSUM tiles (ACT the X
            # tile, DVE the Y tile); output streamed in half-row-block
            # pieces on the sync HWDGE queue; the last row block emits in
            # quarters so the final transfer is small ----
            for rb in range(NW, NRB - 1):
                for h in range(2):
                    stx, sty = half_mm(rb, 4 * h)
                    drain(rb, stx, 4 * h, raw=False)
                    drain(rb, sty, 4 * h + 2, raw=True)
                # one full-row 1MB emit per rb: 8KB-contiguous DRAM rows,
                # rings alternated by rb parity
                emit_out(rb, 0, 8, nc.sync if rb % 2 == 0 else nc.gpsimd)
            # last row block: h0 as usual, then two 1000-col units with
            # 500-col split drains so the final drain+DMA chain is short
            rb = NRB - 1
            stx, sty = half_mm(rb, 0)
            drain(rb, stx, 0, raw=False)
            drain(rb, sty, 2, raw=True)
            emit_out(rb, 0, 4, nc.gpsimd)
            for c0 in (4, 6):
                stqx = ps.tile([128, 2, 512], fp32, tag="stripe",
                               name=f"lqx{c0}")
                stqy = ps.tile([128, 2, 512], fp32, tag="stripe",
                               name=f"lqy{c0}")
                for kp in range(2):
                    mm(stqx, rb, c0, 0, kp)
                    mm(stqy, rb, c0, 1, kp)
                if all_bias:
                    nc.tensor.matmul(
                        stqx[:, 0, 0:500], ones_sb[:, :],
                        bias_sb[:, c0 * 500:(c0 + 1) * 500],
                        start=False, stop=True)
                    nc.tensor.matmul(
                        stqy[:, 1, 0:500], ones_sb[:, :],
                        bias_sb[:, (c0 + 1) * 500:(c0 + 2) * 500],
                        start=False, stop=True)
                ot = get_ot(rb)
                nc.scalar.activation(ot[:, c0 * 500:(c0 + 1) * 500],
                                     stqx[:, 0, 0:500], AF.Exp,
                                     scale=1.0 / (SH * SW),
                                     bias=lnb_sb[:, rb:rb + 1])
                nc.vector.tensor_copy(ot[:, (c0 + 1) * 500:(c0 + 2) * 500],
                                      stqy[:, 1, 0:500])
                emit_out(rb, c0, c0 + 2, nc.sync)

    nc.compile()
    return nc


def _get_nc(all_bias: bool):
    key = ("nc", all_bias)
    if key not in _cache:
        _cache[key] = _build(all_bias)
    return _cache[key]


def kernel(hidden, attn, src_map, W, b, Wc, bc):
    from concourse.bass_utils import run_bass_kernel_spmd

    hidden = np.asarray(hidden, dtype=np.float32)
    attn = np.asarray(attn, dtype=np.float32)
    src_map = np.asarray(src_map, dtype=np.float32)
    W = np.asarray(W, dtype=np.float32)
    b = np.asarray(b, dtype=np.float32)
    Wc = np.asarray(Wc, dtype=np.float32)
    bc = np.asarray(bc, dtype=np.float32)

    all_bias = bool(np.any(b != 0.0))

    # host prologue: p_copy (tiny matvec) and the per-row ACT bias ln(1-p)
    z = hidden.astype(np.float64) @ Wc.astype(np.float64) + bc.astype(np.float64)
    p = 1.0 / (1.0 + np.exp(-z))                         # [R, 1]
    one_m_p = (1.0 - p).reshape(-1)                      # [R]
    lnb = np.log(one_m_p).reshape(NRB, 128).T.astype(np.float32)  # [128, NRB]
    lnb = np.ascontiguousarray(lnb)

    # hT packed per row-chunk: [128, sum(kch*len)], chunk layout [p][kk][r]
    hT_f8 = np.clip(hidden.T * SH, -240.0, 240.0).astype(f8)  # [512, 2048]
    parts = []
    for r0, r1 in HT_CH:
        parts.append(hT_f8[:, r0:r1].reshape(KCH, 128, r1 - r0)
                     .transpose(1, 0, 2).reshape(128, -1))
    hTp = np.ascontiguousarray(np.concatenate(parts, axis=1))  # [128, 8192]

    attnS = attn * p.astype(np.float32)                  # [R, S] attn * p_copy

    nc = _get_nc(all_bias)

    in_maps = []
    for k in range(NC):
        # W shard packed [p][q][kk][j-pad-512]: chunk q = 500 vocab cols
        Wk8 = np.clip(W[:, k * VS:(k + 1) * VS] * SW, -240.0, 240.0).astype(f8)
        Wk8 = Wk8.reshape(KCH, 128, 8, 500).transpose(1, 2, 0, 3)  # [p,q,c,j]
        Wp = np.zeros((128, 8, KCH, 512), dtype=f8)
        Wp[:, :, :, 0:500] = Wk8
        Wp = Wp.reshape(128, 8 * KCH * 512)

        # copy-path shard: batches 4k..4k+3, packed col j = bb*64 + t;
        # attnT and srcmap packed into one [100, SCH, 656] buffer
        # (cpin[p, c] = row c*100+p of the [S, ...] layouts)
        rows = np.array([[t * 32 + 4 * k + bb for t in range(T)] for bb in range(4)])
        rows_flat = rows.reshape(-1)
        attnT_k = attnS[rows_flat, :].T.astype(bf16)                         # [400, 256]
        srcmap_k = src_map[:, 4 * k:4 * k + 4, :].reshape(S, 4 * C).astype(bf16)
        cpin_k = np.empty((100, SCH, 256 + 4 * C), dtype=bf16)
        cpin_k[:, :, 0:256] = attnT_k.reshape(SCH, 100, 256).transpose(1, 0, 2)
        cpin_k[:, :, 256:] = srcmap_k.reshape(SCH, 100, 4 * C).transpose(1, 0, 2)
        cpin_k = np.ascontiguousarray(cpin_k.reshape(100, -1))

        im = {"hTp": hTp, "Wp": Wp, "lnb": lnb, "cpin": cpin_k}
        if all_bias:
            bias_k = b[k * VS:(k + 1) * VS].astype(np.float64)
            if k == 0:
                bias_k = bias_k.copy()
                bias_k[PAD_IDX] += NEG_INF
            # the PSUM logits carry the SH*SW pre-scale; match it so the
            # ACT scale 1/(SH*SW) recovers logit + b
            im["biask"] = (bias_k * SH * SW).astype(bf16)[None, :]           # [1, 4000]
        in_maps.append(im)

    global _last_in_maps
    _last_in_maps = in_maps
    res = run_bass_kernel_spmd(nc, in_maps, core_ids=list(range(NC))).results

    # host epilogue: the DVE-drained 1000-col blocks hold raw bf16 logits
    # *SH*SW -> exp them here; then finish the softmax denominator and
    # normalize while upcasting.
    full = np.empty((R, V + C), dtype=np.float32)
    for k in range(NC):
        full[:, k * VS:(k + 1) * VS] = res[k]["out"]
    for rb in range(NRB):
        r0, r1 = rb * 128, (rb + 1) * 128
        for s in _dve_blocks(rb):
            for k in range(NC):
                blk = full[r0:r1, k * VS + s * 500:k * VS + (s + 1) * 500]
                np.exp(blk * (1.0 / (SH * SW)), out=blk)
                blk *= one_m_p[r0:r1, None].astype(np.float32)

    s_row = full[:, :V].sum(axis=1, dtype=np.float64)    # (1-p) * (Z + e_pad)
    # remove the PAD column's contribution (device computed exp there too;
    # it is 0 in the all_bias build, so the same formula covers both)
    scale = (one_m_p / (s_row - full[:, PAD_IDX])).astype(np.float32)
    full[:, :V] *= scale[:, None]
    full[:, PAD_IDX] = 0.0

    t_idx = np.arange(T) * 32
    for k in range(NC):
        cp = res[k]["cp"].reshape(T, 4, C)
        for bb in range(4):
            full[t_idx + 4 * k + bb, V:] = cp[:, bb, :]
    return full
